# revision 21
# baseline (speedup 1.0000x reference)
"""Trainium2 Bass kernel for nn_CrossAttention (LN -> Q/K/V proj -> per-position
per-head dot-product gate, no softmax).

Strategy (v3.1, bf16 + DMA-xbar transpose + deep software pipeline):
  - Data-parallel over batch: 8 cores x 2 batches each (4096 token rows/core).
  - Host folds LayerNorm affine params into projection weights (q side also
    takes the 1/sqrt(d)=1/8 gate scale), converts everything to bf16, and
    packs x|xf into one [NTOK, 1280] tensor: ONE load DMA + ONE xbar
    transpose per 128-token chunk.
  - Work placement per chunk (steady state):
      PE   : 16 bf16 matmuls N=512 (~3.4us)  <- bottleneck
      DVE  : bn_stats/aggr, reciprocal, xf-normalize, per-head reduce
      ACT  : sqrt, x-normalize (Identity w/ AP scale+bias), PSUM drains
             (q drained with scale=-1 so (1-w)*q becomes (w-1)*qneg)
      Pool : pp = qneg*k, y1 = (w-1)*qneg, y2 = w*v (broadcast APs)
      SP   : load / xbar-transpose / store DMA issue
  - Stages are emitted in skewed waves (one pipeline stage per cross-engine
    hop) so each engine queue executes in dependency-arrival order -- no
    head-of-line blocking, PE stays continuously fed at max p-state.
"""

import math
from contextlib import ExitStack

import numpy as np
import ml_dtypes

import concourse.bacc as bacc
import concourse.bass as bass
import concourse.tile as tile
from concourse import mybir
from concourse.bass_utils import run_bass_kernel_spmd

F32 = mybir.dt.float32
BF16 = mybir.dt.bfloat16
AF = mybir.ActivationFunctionType
ALU = mybir.AluOpType

# Problem shapes (hardcoded per spec)
B, T, D, L, HD = 16, 2048, 512, 768, 512
H, DH = 8, 64
EPS = 1e-5
NCORES = 8
B_LOC = B // NCORES          # 2
NTOK = B_LOC * T             # 4096 token rows per core
P = 128
NCHUNK = NTOK // P           # 32
DC = D // P                  # 4 contraction chunks for x
LC = L // P                  # 6 contraction chunks for xf
W_ALL = D + L                # 1280 packed width
CC = W_ALL // P              # 10 transposed chunks

BF16_NP = ml_dtypes.bfloat16


def build_program(with_bias: bool):
    nc = bacc.Bacc(
        "TRN2",
        target_bir_lowering=False,
        debug=False,
        enable_asserts=False,
        num_devices=NCORES,
    )

    xx_d = nc.dram_tensor("xx", [NTOK, W_ALL], BF16, kind="ExternalInput").ap()
    wq_d = nc.dram_tensor("wq", [P, DC, HD], BF16, kind="ExternalInput").ap()
    wk_d = nc.dram_tensor("wk", [P, LC, HD], BF16, kind="ExternalInput").ap()
    wv_d = nc.dram_tensor("wv", [P, LC, HD], BF16, kind="ExternalInput").ap()
    if with_bias:
        bq_d = nc.dram_tensor("bq", [1, HD], BF16, kind="ExternalInput").ap()
        bk_d = nc.dram_tensor("bk", [1, HD], BF16, kind="ExternalInput").ap()
        bv_d = nc.dram_tensor("bv", [1, HD], BF16, kind="ExternalInput").ap()
    y_d = nc.dram_tensor("y", [NTOK, 2 * HD], BF16, kind="ExternalOutput").ap()

    with tile.TileContext(nc) as tc, ExitStack() as ctx:
        consts = ctx.enter_context(tc.tile_pool(name="consts", bufs=1))
        loads = ctx.enter_context(tc.tile_pool(name="loads", bufs=7))
        mids = ctx.enter_context(tc.tile_pool(name="mids", bufs=5))
        qkv = ctx.enter_context(tc.tile_pool(name="qkv", bufs=5))
        small = ctx.enter_context(tc.tile_pool(name="small", bufs=7))
        outs = ctx.enter_context(tc.tile_pool(name="outs", bufs=5))
        gpq = ctx.enter_context(tc.tile_pool(name="gpq", bufs=2, space="PSUM"))
        gpk = ctx.enter_context(tc.tile_pool(name="gpk", bufs=3, space="PSUM"))
        gpv = ctx.enter_context(tc.tile_pool(name="gpv", bufs=3, space="PSUM"))

        # Resident constants
        wq_s = consts.tile([P, DC, HD], BF16)
        nc.sync.dma_start(out=wq_s, in_=wq_d)
        wk_s = consts.tile([P, LC, HD], BF16)
        nc.sync.dma_start(out=wk_s, in_=wk_d)
        wv_s = consts.tile([P, LC, HD], BF16)
        nc.sync.dma_start(out=wv_s, in_=wv_d)
        eps_t = consts.tile([P, 1], F32)
        nc.vector.memset(eps_t, EPS)
        if with_bias:
            ones_row = consts.tile([1, P], BF16)
            nc.vector.memset(ones_row, 1.0)
            bq_s = consts.tile([1, HD], BF16)
            nc.sync.dma_start(out=bq_s, in_=bq_d)
            bk_s = consts.tile([1, HD], BF16)
            nc.sync.dma_start(out=bk_s, in_=bk_d)
            bv_s = consts.tile([1, HD], BF16)
            nc.sync.dma_start(out=bv_s, in_=bv_d)

        tk: dict = {}

        def s_load(i):
            xx_t = loads.tile([P, W_ALL], BF16, tag="xx_t")
            tk["xx", i] = xx_t
            nc.sync.dma_start(out=xx_t, in_=xx_d[bass.ts(i, P), :])

        def s_stats(i):
            xx_t = tk["xx", i]
            st_x = small.tile([P, 6], F32, tag="st_x")
            nc.vector.bn_stats(st_x, xx_t[:, 0:D])
            st_f = small.tile([P, 2, 6], F32, tag="st_f")
            nc.vector.bn_stats(st_f[:, 0, :], xx_t[:, D : D + L // 2])
            nc.vector.bn_stats(st_f[:, 1, :], xx_t[:, D + L // 2 : W_ALL])
            # mvb[:, 0, :] = (mean_x, var_x) ; mvb[:, 1, :] = (mean_f, var_f)
            mvb = small.tile([P, 2, 2], F32, tag="mvb")
            tk["mvb", i] = mvb
            nc.vector.bn_aggr(mvb[:, 0, :], st_x)
            nc.vector.bn_aggr(mvb[:, 1, :], st_f)

        def s_rsqrt(i):
            # rs2 = 1/sqrt(var + eps) for both tensors in one ACT op
            # (var+eps > 0 so Abs_reciprocal_sqrt == rsqrt)
            rs2 = small.tile([P, 2], F32, tag="rs2")
            tk["rs2", i] = rs2
            nc.scalar.activation(
                rs2,
                tk["mvb", i][:, :, 1:2],
                AF.Abs_reciprocal_sqrt,
                bias=eps_t,
                scale=1.0,
            )

        def s_normf(i):
            rs2 = tk["rs2", i]
            nmxr = small.tile([P, 1], F32, tag="nmxr")
            tk["nmxr", i] = nmxr
            nc.vector.tensor_scalar(
                out=nmxr,
                in0=tk["mvb", i][:, 0, 0:1],
                scalar1=rs2[:, 0:1],
                scalar2=-1.0,
                op0=ALU.mult,
                op1=ALU.mult,
            )
            xhh = mids.tile([P, W_ALL], BF16, tag="xhh")
            tk["xhh", i] = xhh
            nc.vector.tensor_scalar(
                out=xhh[:, D:W_ALL],
                in0=tk["xx", i][:, D:W_ALL],
                scalar1=tk["mvb", i][:, 1, 0:1],
                scalar2=rs2[:, 1:2],
                op0=ALU.subtract,
                op1=ALU.mult,
            )

        def s_normx(i):
            # x-normalize on ACT: Identity(x * rsx + (-mx*rsx))
            nc.scalar.activation(
                tk["xhh", i][:, 0:D],
                tk["xx", i][:, 0:D],
                AF.Identity,
                bias=tk["nmxr", i],
                scale=tk["rs2", i][:, 0:1],
            )

        def s_transpose(i):
            xxT = mids.tile([P, CC, P], BF16, tag="xxT")
            tk["xxT", i] = xxT
            nc.sync.dma_start(out=xxT, in_=tk["xhh", i], transpose=True)

        def s_matmul(i):
            xxT = tk["xxT", i]
            gq = gpq.tile([P, HD], F32, tag="gq")
            tk["gq", i] = gq
            for c in range(DC):
                nc.tensor.matmul(
                    gq,
                    lhsT=xxT[:, c, :],
                    rhs=wq_s[:, c, :],
                    start=(c == 0),
                    stop=(c == DC - 1 and not with_bias),
                )
            if with_bias:
                nc.tensor.matmul(gq, lhsT=ones_row, rhs=bq_s, start=False, stop=True)
            gk = gpk.tile([P, HD], F32, tag="gk")
            tk["gk", i] = gk
            gv = gpv.tile([P, HD], F32, tag="gv")
            tk["gv", i] = gv
            for c in range(LC):
                nc.tensor.matmul(
                    gk,
                    lhsT=xxT[:, DC + c, :],
                    rhs=wk_s[:, c, :],
                    start=(c == 0),
                    stop=(c == LC - 1 and not with_bias),
                )
                nc.tensor.matmul(
                    gv,
                    lhsT=xxT[:, DC + c, :],
                    rhs=wv_s[:, c, :],
                    start=(c == 0),
                    stop=(c == LC - 1 and not with_bias),
                )
            if with_bias:
                nc.tensor.matmul(gk, lhsT=ones_row, rhs=bk_s, start=False, stop=True)
                nc.tensor.matmul(gv, lhsT=ones_row, rhs=bv_s, start=False, stop=True)

        def s_drain(i):
            # qn first: it frees the PSUM bank gating the next q-matmuls.
            # q drained negated: y1 = (1-w)*q == (w-1)*qneg
            qn = qkv.tile([P, HD], BF16, tag="qn")
            tk["qn", i] = qn
            nc.scalar.mul(qn, tk["gq", i], -1.0)
            kvs = qkv.tile([P, 2, HD], BF16, tag="kvs")
            tk["kvs", i] = kvs
            nc.scalar.copy(kvs[:, 0, :], tk["gk", i])
            nc.scalar.copy(kvs[:, 1, :], tk["gv", i])

        def s_ppred(i):
            # pp + per-head reduce, both DVE (intra-engine chain, no hop)
            pp = mids.tile([P, HD], BF16, tag="pp")
            nc.vector.tensor_tensor(
                out=pp, in0=tk["qn", i], in1=tk["kvs", i][:, 0, :], op=ALU.mult
            )
            w = small.tile([P, H], F32, tag="w")
            tk["w", i] = w
            nc.vector.tensor_reduce(
                out=w,
                in_=pp.rearrange("p (h d) -> p h d", h=H),
                axis=mybir.AxisListType.X,
                op=ALU.add,
                negate=True,
            )

        def s_tm1(i):
            # tm1 = w - 1, so y1 = (1-w)*q == tm1 * qneg  (Pool)
            tm1 = small.tile([P, H], F32, tag="tm1")
            tk["tm1", i] = tm1
            nc.gpsimd.tensor_scalar_add(tm1, tk["w", i], -1.0)

        def s_gate(i):
            w = tk["w", i]
            tm1 = tk["tm1", i]
            qn = tk["qn", i]
            kvs = tk["kvs", i]
            w_bcast = bass.AP(
                tensor=w.tensor, offset=w.offset, ap=[w.ap[0], w.ap[1], [0, DH]]
            )
            tm1_bcast = bass.AP(
                tensor=tm1.tensor, offset=tm1.offset,
                ap=[tm1.ap[0], tm1.ap[1], [0, DH]],
            )
            yt = outs.tile([P, 2, HD], BF16, tag="yt")
            tk["yt", i] = yt
            nc.gpsimd.tensor_tensor(
                out=yt[:, 0, :].rearrange("p (h d) -> p h d", h=H),
                in0=tm1_bcast,
                in1=qn.rearrange("p (h d) -> p h d", h=H),
                op=ALU.mult,
            )
            nc.gpsimd.tensor_tensor(
                out=yt[:, 1, :].rearrange("p (h d) -> p h d", h=H),
                in0=w_bcast,
                in1=kvs[:, 1, :].rearrange("p (h d) -> p h d", h=H),
                op=ALU.mult,
            )

        def s_store(i):
            nc.gpsimd.dma_start(out=y_d[bass.ts(i, P), :], in_=tk["yt", i])

        # (skew, fn): chunk i's stage fn runs in wave i+skew.  Skews are
        # chosen so every cross-engine dependency is >=1 wave old (no
        # intra-wave ping-pong) and DMA stages get 2 waves of latency.
        # List order = within-wave emission order = per-engine queue order:
        # ACT services the PSUM drains (which gate the next PE burst) before
        # its stats/normalize work; SP issues the load before the transpose.
        stages = [
            (0, s_load),
            (9, s_drain),
            (8, s_matmul),
            (2, s_stats),
            (3, s_rsqrt),
            (4, s_normf),
            (5, s_normx),
            (6, s_transpose),
            (10, s_ppred),
            (11, s_tm1),
            (12, s_gate),
            (14, s_store),
        ]
        max_skew = max(s for s, _ in stages)
        for wave in range(NCHUNK + max_skew):
            for s, fn in stages:
                i = wave - s
                if 0 <= i < NCHUNK:
                    fn(i)

    nc.compile()
    return nc


_PROGRAM_CACHE: dict = {}


def _get_program(with_bias: bool):
    if with_bias not in _PROGRAM_CACHE:
        _PROGRAM_CACHE[with_bias] = build_program(with_bias)
    return _PROGRAM_CACHE[with_bias]


def _prep_host(inputs):
    norm_w = np.asarray(inputs["norm_w"], np.float32)
    norm_b = np.asarray(inputs["norm_b"], np.float32)
    tnorm_w = np.asarray(inputs["tnorm_w"], np.float32)
    tnorm_b = np.asarray(inputs["tnorm_b"], np.float32)
    Wq = np.asarray(inputs["Wq"], np.float32)
    Wk = np.asarray(inputs["Wk"], np.float32)
    Wv = np.asarray(inputs["Wv"], np.float32)

    # gate scale 1/sqrt(d)=1/8 folded into the K side so q stays unscaled
    # (y1 = (1-w)*q is computed directly from the drained q)
    scale_q = 1.0 / math.sqrt(DH)
    wq_eff = norm_w[:, None] * Wq.T                  # [D, HD]
    wk_eff = (tnorm_w[:, None] * Wk.T) * scale_q     # [L, HD]
    wv_eff = tnorm_w[:, None] * Wv.T                 # [L, HD]
    bq = norm_b @ Wq.T                               # [HD]
    bk = (tnorm_b @ Wk.T) * scale_q
    bv = tnorm_b @ Wv.T

    # [D, HD] -> [P, DC, HD]: partition p holds rows {c*128+p}
    wq_h = np.ascontiguousarray(
        wq_eff.reshape(DC, P, HD).transpose(1, 0, 2)
    ).astype(BF16_NP)
    wk_h = np.ascontiguousarray(
        wk_eff.reshape(LC, P, HD).transpose(1, 0, 2)
    ).astype(BF16_NP)
    wv_h = np.ascontiguousarray(
        wv_eff.reshape(LC, P, HD).transpose(1, 0, 2)
    ).astype(BF16_NP)
    with_bias = bool(np.any(norm_b) or np.any(tnorm_b))
    return wq_h, wk_h, wv_h, bq, bk, bv, with_bias


def make_in_maps(inputs):
    x = np.asarray(inputs["x"], np.float32)
    xf = np.asarray(inputs["xf"], np.float32)
    wq_h, wk_h, wv_h, bq, bk, bv, with_bias = _prep_host(inputs)
    xx = np.concatenate(
        [x.astype(BF16_NP).reshape(B, T, D), xf.astype(BF16_NP).reshape(B, T, L)],
        axis=2,
    )  # [B, T, 1280] bf16

    in_maps = []
    for i in range(NCORES):
        m = {
            "xx": np.ascontiguousarray(
                xx[i * B_LOC : (i + 1) * B_LOC].reshape(NTOK, W_ALL)
            ),
            "wq": wq_h,
            "wk": wk_h,
            "wv": wv_h,
        }
        if with_bias:
            m["bq"] = bq.reshape(1, HD).astype(BF16_NP)
            m["bk"] = bk.reshape(1, HD).astype(BF16_NP)
            m["bv"] = bv.reshape(1, HD).astype(BF16_NP)
        in_maps.append(m)
    return in_maps, with_bias


def split_y(y_flat):
    """[NTOK, 2*HD] fused output -> (y1, y2) each [B_LOC, T, HD] f32."""
    y = np.asarray(y_flat).reshape(B_LOC, T, 2, HD).astype(np.float32)
    return y[:, :, 0, :], y[:, :, 1, :]


def kernel(**inputs):
    in_maps, with_bias = make_in_maps(inputs)
    nc = _get_program(with_bias)
    res = run_bass_kernel_spmd(nc, in_maps, core_ids=list(range(NCORES)))
    y1_parts = []
    y2_parts = []
    for r in res.results:
        y1_c, y2_c = split_y(r["y"])
        y1_parts.append(y1_c)
        y2_parts.append(y2_c)
    return (np.concatenate(y1_parts, axis=0), np.concatenate(y2_parts, axis=0))
